# revision 6
# baseline (speedup 1.0000x reference)
"""GCNConv (D^-1/2 A D^-1/2 X W + b) on 8 Trainium2 NeuronCores.

Strategy (row-sharded over nodes, hint-compliant):
  - each core owns a [1024, 8192] row block of the adjacency and the matching
    rows of input_feature; weight/bias replicated.
  - support = X @ W computed per-core on its rows, AllGathered (bf16).
  - the adjacency block is streamed ONCE from HBM with an f32->bf16 casting
    DMA; per 128-row m-tile we accumulate rowsums (degree), and transpose the
    block on the TensorEngine (regular matmul against an identity: native
    tile as the stationary operand) into a resident bf16 A^T in SBUF.
  - d = (deg + l)^-1/2 is AllGathered in 8 pipelined 128-row chunks (one per
    m-tile) so the main matmul A @ (d * support) can start while the stream
    is still running.  Output is computed transposed (out^T accumulated as
    SV-chunk-stationary x A^T-moving), scaled by d_m (f32) + bias in the
    epilogue, and un-transposed on the host during unsharding.
"""
import sys
sys.path.insert(0, "/opt/trn_rl_repo")
from contextlib import ExitStack

import numpy as np

import concourse.bass as bass
import concourse.bacc as bacc
import concourse.tile as tile
import concourse.bass_utils as bass_utils
import concourse.mybir as mybir

N_CORES = 8
N = 8192
DIN = 256
DOUT = 256
P = 128
M_LOC = N // N_CORES          # 1024 rows per core
MT = M_LOC // P               # 8 m-tiles per core
KT = N // P                   # 64 k-tiles global
CHUNK = 4096                  # k-chunk per streaming DMA
NCH = N // CHUNK              # 2 chunks per m-tile
SUB = CHUNK // P              # 32 k-subtiles per chunk
F32 = mybir.dt.float32
BF16 = mybir.dt.bfloat16
RG = [list(range(N_CORES))]
Alu = mybir.AluOpType
AxX = mybir.AxisListType.X


def _emit_body(nc, tc, pools, consts, rep):
    (natp, supp, xtp, atpp, svp, dslp, dtp, stagep, tpp, mmp, auxp,
     dram) = pools
    (ident, wb, bias_sb, lv, ones_row, a, x, w, bias, out_t) = consts
    R = f"r{rep}_"

    # ---- DRAM bounce buffers for collectives ----
    sag_in = dram.tile([M_LOC, DOUT], BF16, tag="sag_in", name=R + "sag_in")
    sag_out = dram.tile([N, DOUT], BF16, addr_space="Shared", tag="sag_out",
                        name=R + "sag_out")
    dag_in = [dram.tile([P, 1], F32, tag=f"dag_in{i}", name=R + f"dag_in{i}")
              for i in range(MT)]
    dag_out = [dram.tile([P * N_CORES, 1], F32, addr_space="Shared",
                         tag=f"dag_out{i}", name=R + f"dag_out{i}")
               for i in range(MT)]

    # ---- support = X @ W (bf16), AllGather ----
    xt = [xtp.tile([P, M_LOC], BF16, tag="xt", name=R + f"xt{dt}")
          for dt in range(DIN // P)]
    for i in range(MT):
        xb = supp.tile([P, DIN], BF16, tag="sup", name=R + f"xb{i}")
        nc.gpsimd.dma_start(xb[:], x.ap()[i * P:(i + 1) * P, :])
        for dt in range(DIN // P):
            ps = tpp.tile([P, P], F32, tag="tp", name=R + f"xps{i}_{dt}")
            nc.tensor.matmul(ps[:], xb[:, dt * P:(dt + 1) * P], ident[:],
                             start=True, stop=True)
            nc.vector.tensor_copy(xt[dt][:, i * P:(i + 1) * P], ps[:])
    for i in range(MT):
        sps = auxp.tile([P, DOUT], F32, tag="aux", name=R + f"sps{i}")
        for dt in range(DIN // P):
            nc.tensor.matmul(sps[:], xt[dt][:, i * P:(i + 1) * P], wb[dt][:],
                             start=(dt == 0), stop=(dt == DIN // P - 1))
        sst = supp.tile([P, DOUT], BF16, tag="sup", name=R + f"sst{i}")
        nc.scalar.copy(sst[:], sps[:])
        nc.sync.dma_start(sag_in[i * P:(i + 1) * P, :], sst[:])
    nc.gpsimd.collective_compute(
        "AllGather", Alu.bypass, replica_groups=RG,
        ins=[sag_in.opt()], outs=[sag_out.opt()])

    # ---- resident transposed adjacency (bf16) and SV tiles ----
    atp = {}
    for t in range(KT):
        for h in range(2):
            atp[(t, h)] = atpp.tile([P, 512], BF16, tag="atp",
                                    name=R + f"atp_{t}_{h}")
    sv = [svp.tile([P, DOUT], BF16, tag="sv", name=R + f"sv{t}")
          for t in range(KT)]

    par = dtp.tile([P, MT * NCH], F32, tag="par", name=R + "par")
    dcols = dtp.tile([P, MT], F32, tag="dcols", name=R + "dcols")
    dsl = [dslp.tile([P, N_CORES], F32, tag="dsl", name=R + f"dsl{i}")
           for i in range(MT)]

    # ---- main stream over the adjacency block ----
    for i in range(MT):
        for j in range(NCH):
            nat = natp.tile([P, CHUNK], BF16, tag="nat", name=R + f"nat{i}_{j}")
            nc.gpsimd.dma_start(
                nat[:], a.ap()[i * P:(i + 1) * P, j * CHUNK:(j + 1) * CHUNK])
            c = i * NCH + j
            nc.vector.tensor_reduce(par[:, c:c + 1], nat[:], axis=AxX,
                                    op=Alu.add)
            for s in range(SUB):
                t = j * SUB + s
                ps = tpp.tile([P, P], F32, tag="tp", name=R + f"tps{i}_{t}")
                nc.tensor.matmul(ps[:], nat[:, s * P:(s + 1) * P], ident[:],
                                 start=True, stop=True)
                dst = atp[(t, i // 4)][:, (i % 4) * P:(i % 4 + 1) * P]
                if t % 2 == 0:
                    nc.vector.tensor_copy(dst, ps[:])
                else:
                    nc.scalar.copy(dst, ps[:])
        # degree -> d for this m-tile
        deg = dtp.tile([P, 1], F32, tag="deg", bufs=2, name=R + f"deg{i}")
        nc.vector.tensor_add(deg[:], par[:, i * NCH:i * NCH + 1],
                             par[:, i * NCH + 1:i * NCH + 2])
        deg2 = dtp.tile([P, 1], F32, tag="deg2", bufs=2, name=R + f"deg2{i}")
        nc.vector.tensor_scalar_add(deg2[:], deg[:], lv[:])
        s0 = dtp.tile([P, 1], F32, tag="s0", bufs=2, name=R + f"s0{i}")
        nc.scalar.sqrt(s0[:], deg2[:])
        r0 = dtp.tile([P, 1], F32, tag="r0", bufs=2, name=R + f"r0{i}")
        nc.vector.reciprocal(r0[:], s0[:])
        # one Newton step: d = r0 * (1.5 - 0.5 * deg2 * r0^2)
        t1 = dtp.tile([P, 1], F32, tag="t1", bufs=2, name=R + f"t1{i}")
        nc.vector.tensor_mul(t1[:], r0[:], r0[:])
        t2 = dtp.tile([P, 1], F32, tag="t2", bufs=2, name=R + f"t2{i}")
        nc.vector.tensor_mul(t2[:], t1[:], deg2[:])
        t3 = dtp.tile([P, 1], F32, tag="t3", bufs=2, name=R + f"t3{i}")
        nc.vector.tensor_scalar(t3[:], t2[:], -0.5, 1.5, op0=Alu.mult,
                                op1=Alu.add)
        nc.vector.tensor_mul(dcols[:, i:i + 1], r0[:], t3[:])
        nc.sync.dma_start(dag_in[i][:], dcols[:, i:i + 1])
        nc.gpsimd.collective_compute(
            "AllGather", Alu.bypass, replica_groups=RG,
            ins=[dag_in[i].opt()], outs=[dag_out[i].opt()])
        nc.sync.dma_start(
            dsl[i][:], dag_out[i][:].rearrange("(r p) o -> p (r o)", p=P))
        for r in range(N_CORES):
            t = r * MT + i
            stile = supp.tile([P, DOUT], BF16, tag="sup",
                              name=R + f"stile{t}")
            nc.sync.dma_start(stile[:], sag_out[t * P:(t + 1) * P, :])
            nc.vector.tensor_scalar_mul(sv[t][:], stile[:],
                                        dsl[i][:, r:r + 1])

    # ---- main matmul: out^T[n, m] accumulated over k (AG-phase order) ----
    mmps = [[mmp.tile([P, 512], F32, tag="mmps", name=R + f"mmps_{nn}_{hh}")
             for hh in range(2)] for nn in range(2)]
    for phase in range(MT):
        for r in range(N_CORES):
            t = r * MT + phase
            first = (phase == 0 and r == 0)
            last = (phase == MT - 1 and r == N_CORES - 1)
            for nchk in range(2):
                for h in range(2):
                    nc.tensor.matmul(
                        mmps[nchk][h][:],
                        sv[t][:, nchk * P:(nchk + 1) * P],
                        atp[(t, h)][:],
                        start=first, stop=last)

    # ---- epilogue: scale by d_m, add bias, store out^T ----
    d_row = dtp.tile([1, M_LOC], F32, tag="d_row", name=R + "d_row")
    for i in range(MT):
        nc.sync.dma_start(d_row[0:1, i * P:(i + 1) * P], dcols[:, i:i + 1])
    dbc = dtp.tile([P, M_LOC], F32, tag="dbc", name=R + "dbc")
    for h in range(2):
        bc = auxp.tile([P, 512], F32, tag="aux", name=R + f"bc{h}")
        nc.tensor.matmul(bc[:], ones_row[:],
                         d_row[0:1, h * 512:(h + 1) * 512],
                         start=True, stop=True)
        nc.vector.tensor_copy(dbc[:, h * 512:(h + 1) * 512], bc[:])
    for nchk in range(2):
        for h in range(2):
            st1 = stagep.tile([P, 512], F32, tag="stage",
                              name=R + f"st1_{nchk}_{h}")
            nc.vector.tensor_mul(st1[:], mmps[nchk][h][:],
                                 dbc[:, h * 512:(h + 1) * 512])
            st2 = stagep.tile([P, 512], F32, tag="stage",
                              name=R + f"st2_{nchk}_{h}")
            nc.vector.tensor_scalar_add(st2[:], st1[:], bias_sb[nchk][:])
            nc.sync.dma_start(
                out_t.ap()[nchk * P:(nchk + 1) * P,
                           h * 512:(h + 1) * 512], st2[:])


def build(repeat=1):
    nc = bacc.Bacc("TRN2", target_bir_lowering=False, debug=False,
                   num_devices=N_CORES)
    a = nc.dram_tensor("a", [M_LOC, N], F32, kind="ExternalInput")
    x = nc.dram_tensor("x", [M_LOC, DIN], F32, kind="ExternalInput")
    w = nc.dram_tensor("w", [DIN, DOUT], F32, kind="ExternalInput")
    bias = nc.dram_tensor("bias", [DOUT], F32, kind="ExternalInput")
    lvec = nc.dram_tensor("lvec", [P, 1], F32, kind="ExternalInput")
    out_t = nc.dram_tensor("out_t", [DOUT, M_LOC], F32, kind="ExternalOutput")

    with tile.TileContext(nc) as tc, ExitStack() as ctx:
        cpool = ctx.enter_context(tc.tile_pool(name="cpool", bufs=1))
        natp = ctx.enter_context(tc.tile_pool(name="natp", bufs=2))
        supp = ctx.enter_context(tc.tile_pool(name="supp", bufs=8))
        xtp = ctx.enter_context(tc.tile_pool(name="xtp", bufs=2))
        atpp = ctx.enter_context(tc.tile_pool(name="atpp", bufs=2 * KT))
        svp = ctx.enter_context(tc.tile_pool(name="svp", bufs=KT))
        dslp = ctx.enter_context(tc.tile_pool(name="dslp", bufs=MT))
        dtp = ctx.enter_context(tc.tile_pool(name="dtp", bufs=1))
        stagep = ctx.enter_context(tc.tile_pool(name="stagep", bufs=2))
        tpp = ctx.enter_context(tc.tile_pool(name="tpp", bufs=3, space="PSUM"))
        mmp = ctx.enter_context(tc.tile_pool(name="mmp", bufs=4, space="PSUM"))
        auxp = ctx.enter_context(tc.tile_pool(name="auxp", bufs=1,
                                              space="PSUM"))
        dram = ctx.enter_context(tc.tile_pool(name="dram", bufs=1,
                                              space="DRAM"))

        # ---- constants ----
        ones_bf = cpool.tile([P, P], BF16)
        nc.vector.memset(ones_bf[:], 1.0)
        ident = cpool.tile([P, P], BF16)
        nc.gpsimd.affine_select(
            ident[:], ones_bf[:], pattern=[[1, P]],
            compare_op=Alu.is_equal, fill=0.0, base=0, channel_multiplier=-1)
        wb = []
        for dt in range(DIN // P):
            wt = cpool.tile([P, DOUT], BF16, tag=f"wb{dt}", name=f"wb{dt}")
            nc.gpsimd.dma_start(wt[:], w.ap()[dt * P:(dt + 1) * P, :])
            wb.append(wt)
        bias_sb = []
        for nchk in range(DOUT // P):
            bt = cpool.tile([P, 1], F32, tag=f"bias{nchk}", name=f"bias{nchk}")
            nc.sync.dma_start(bt[:], bias.ap()[nchk * P:(nchk + 1) * P])
            bias_sb.append(bt)
        lv = cpool.tile([P, 1], F32, tag="lv")
        nc.sync.dma_start(lv[:], lvec.ap())
        ones_row = cpool.tile([1, P], F32, tag="ones_row")
        nc.vector.memset(ones_row[:], 1.0)

        pools = (natp, supp, xtp, atpp, svp, dslp, dtp, stagep, tpp, mmp,
                 auxp, dram)
        consts = (ident, wb, bias_sb, lv, ones_row, a, x, w, bias, out_t)
        for rep in range(repeat):
            _emit_body(nc, tc, pools, consts, rep)
    nc.compile()
    return nc


def make_in_maps(adjacency, input_feature, weight, bias, l):
    adjacency = np.ascontiguousarray(np.asarray(adjacency, dtype=np.float32))
    input_feature = np.ascontiguousarray(
        np.asarray(input_feature, dtype=np.float32))
    weight = np.ascontiguousarray(np.asarray(weight, dtype=np.float32))
    bias_np = np.ascontiguousarray(np.asarray(bias, dtype=np.float32))
    lval = float(np.asarray(l))
    lv = np.full((P, 1), lval, dtype=np.float32)
    in_maps = []
    for c in range(N_CORES):
        in_maps.append({
            "a": adjacency[c * M_LOC:(c + 1) * M_LOC, :],
            "x": input_feature[c * M_LOC:(c + 1) * M_LOC, :],
            "w": weight,
            "bias": bias_np,
            "lvec": lv,
        })
    return in_maps


_NC_CACHE = None


def kernel(adjacency, input_feature, weight, bias, l):
    global _NC_CACHE
    if _NC_CACHE is None:
        _NC_CACHE = build()
    nc = _NC_CACHE
    in_maps = make_in_maps(adjacency, input_feature, weight, bias, l)
    res = bass_utils.run_bass_kernel_spmd(nc, in_maps,
                                          core_ids=list(range(N_CORES)))
    blocks = [res.results[c]["out_t"].T for c in range(N_CORES)]
    return np.ascontiguousarray(np.concatenate(blocks, axis=0),
                                dtype=np.float32)


if __name__ == "__main__":
    rng = np.random.default_rng(0)
    A = rng.random((N, N), dtype=np.float32)
    X = rng.standard_normal((N, DIN)).astype(np.float32)
    W = (rng.standard_normal((DIN, DOUT)) / np.sqrt(DIN)).astype(np.float32)
    B = np.zeros((DOUT,), dtype=np.float32)
    out = kernel(A, X, W, B, 1)
    deg = A.sum(axis=1) + 1.0
    d = np.where(deg > 0, deg ** -0.5, 0.0).astype(np.float32)
    ref = (A * d[:, None] * d[None, :]) @ (X @ W) + B
    err = np.abs(out - ref)
    rel = np.linalg.norm(out - ref) / np.linalg.norm(ref)
    print(f"max abs err {err.max():.3e}  rel l2 {rel:.3e}")


# revision 19
# speedup vs baseline: 2.0483x; 2.0483x over previous
"""GCNConv (D^-1/2 A D^-1/2 X W + b) on 8 Trainium2 NeuronCores.

Strategy (row-sharded over nodes, per the sharding hint):
  - each core owns a [1024, 8192] row block of the adjacency and the matching
    rows of input_feature; weight/bias replicated.
  - support = X @ W computed per-core on its rows, AllGathered (bf16).
  - the adjacency block is streamed ONCE from HBM with an f32->bf16 casting
    DMA; per 128-row m-tile we accumulate rowsums (degree), and transpose the
    block on the TensorEngine (regular matmul against an identity with the
    native tile as the stationary operand -- runs at warm matmul speed,
    unlike transpose-mode).  Four 128x128 transposes land in one [128,512]
    PSUM bank and move to the resident bf16 A^T with a single wide copy
    (alternating DVE/ACT) -- small per-op copy overhead killed.
  - d = (deg + l)^-1/2 (sqrt + reciprocal + one Newton step) is AllGathered
    in 8 pipelined 128-row chunks (one per m-tile) so the main matmul
    A @ (d * support) overlaps the stream; its k-accumulation is ordered by
    AG phase so no matmul waits on a late chunk.
  - main matmul: stationary = A^T 128x128 slice, moving = SV tile [128,256];
    output accumulates in natural [m,n] layout; epilogue scales by d_m
    (per-partition, f32) and adds a broadcast bias.
"""
import sys
sys.path.insert(0, "/opt/trn_rl_repo")
from contextlib import ExitStack

import numpy as np

import concourse.bass as bass
import concourse.bacc as bacc
import concourse.tile as tile
import concourse.bass_utils as bass_utils
import concourse.mybir as mybir

N_CORES = 8
N = 8192
DIN = 256
DOUT = 256
P = 128
M_LOC = N // N_CORES          # 1024 rows per core
MT = M_LOC // P               # 8 m-tiles per core
KT = N // P                   # 64 k-tiles global
GT = KT // 4                  # 16 k-quad groups
CHUNK = 2048                  # k-chunk per streaming DMA
NCH = N // CHUNK              # 4 chunks per m-tile
NQ = CHUNK // (4 * P)         # 4 k-quads per chunk
F32 = mybir.dt.float32
BF16 = mybir.dt.bfloat16
RG = [list(range(N_CORES))]
VARIANT_B = False
Alu = mybir.AluOpType
ActF = mybir.ActivationFunctionType
AxX = mybir.AxisListType.X


def _emit_body(nc, tc, pools, consts, rep, stage="full", dg=2):
    do_transp = stage in ("transp", "transp_nc", "transp_dve", "transp_act",
                          "coll", "coll_d", "coll_sup", "sv", "full",
                          "mm_nocoll")
    copy_mode = {"transp_nc": "none", "transp_dve": "dve",
                 "transp_act": "act"}.get(stage, "both")
    do_coll = stage in ("coll", "coll_d", "sv", "full")
    do_sup_ag = stage in ("coll", "coll_sup", "sv", "full")
    fake_sv = stage == "mm_nocoll"
    do_sv = stage in ("sv", "full")
    do_mm = stage in ("full", "mm_nocoll")
    (natp, natbp, supp, xtp, atpp, svp, dslp, dtp, stagep, tpp, mmp,
     auxp, dram) = pools
    (ident, wb, bias_bc, lv, a, x, w, bias, out) = consts
    R = f"r{rep}_"

    # ---- DRAM bounce buffers for collectives ----
    sag_in = dram.tile([M_LOC, DOUT], BF16, tag="sag_in", name=R + "sag_in")
    sag_out = dram.tile([N, DOUT], BF16, addr_space="Shared", tag="sag_out",
                        name=R + "sag_out")
    MG = MT // dg                 # m-tiles per d-AG chunk
    dag_in = [dram.tile([P, MG], F32, tag=f"dag_in{i}", name=R + f"dag_in{i}")
              for i in range(dg)]
    dag_out = [dram.tile([P * N_CORES * MG, 1], F32, addr_space="Shared",
                         tag=f"dag_out{i}", name=R + f"dag_out{i}")
               for i in range(dg)]

    # ---- support = X @ W (bf16), AllGather ----
    xt = [xtp.tile([P, M_LOC], BF16, tag="xt", name=R + f"xt{dt}")
          for dt in range(DIN // P)]
    for i in range(MT):
        xb = supp.tile([P, DIN], BF16, tag="sup", name=R + f"xb{i}")
        nc.gpsimd.dma_start(xb[:], x.ap()[i * P:(i + 1) * P, :])
        for dt in range(DIN // P):
            ps = tpp.tile([P, 512], F32, tag="tp", name=R + f"xps{i}_{dt}")
            nc.tensor.matmul(ps[:, 0:P], xb[:, dt * P:(dt + 1) * P], ident[:],
                             start=True, stop=True)
            nc.vector.tensor_copy(xt[dt][:, i * P:(i + 1) * P], ps[:, 0:P])
    for i in range(MT):
        sps_t = auxp.tile([P, 512], F32, tag="tp", name=R + f"sps{i}")
        sps = sps_t[:, 0:DOUT]
        for dt in range(DIN // P):
            nc.tensor.matmul(sps, xt[dt][:, i * P:(i + 1) * P], wb[dt][:],
                             start=(dt == 0), stop=(dt == DIN // P - 1))
        sst = supp.tile([P, DOUT], BF16, tag="sup", name=R + f"sst{i}")
        nc.scalar.copy(sst[:], sps)
        nc.scalar.dma_start(sag_in[i * P:(i + 1) * P, :], sst[:])
    if do_sup_ag:
        nc.gpsimd.collective_compute(
            "AllGather", Alu.bypass, replica_groups=RG,
            ins=[sag_in.opt()], outs=[sag_out.opt()])

    # ---- resident transposed adjacency (bf16, k-quad tiles) and SV ----
    atp = {}
    for g in range(GT):
        for i in range(MT):
            atp[(g, i)] = atpp.tile([P, 512], BF16, tag="atp",
                                    name=R + f"atp_{g}_{i}")
    sv = [svp.tile([P, DOUT], BF16, tag="sv", name=R + f"sv{t}")
          for t in range(KT)]

    par = dtp.tile([P, MT * NCH], F32, tag="par", name=R + "par")
    dcols = dtp.tile([P, MT], F32, tag="dcols", name=R + "dcols")
    dsl = [dslp.tile([P, N_CORES * MG], F32, tag="dsl", name=R + f"dsl{i}")
           for i in range(dg)]

    # ---- main stream over the adjacency block ----
    for i in range(MT):
        for j in range(NCH):
            natf = natp.tile([P, CHUNK], F32, tag="nat", name=R + f"natf{i}_{j}")
            nc.sync.dma_start(
                natf[:], a.ap()[i * P:(i + 1) * P, j * CHUNK:(j + 1) * CHUNK])
            nat = natbp.tile([P, CHUNK], BF16, tag="natb",
                             name=R + f"nat{i}_{j}")
            c = i * NCH + j
            nc.scalar.activation(nat[:], natf[:], ActF.Copy,
                                 accum_out=par[:, c:c + 1])
            for q in (range(NQ) if do_transp else ()):
                g = j * NQ + q
                if VARIANT_B:
                    for u in range(4):
                        s = q * 4 + u
                        psu = tpp.tile([P, P], F32, tag="tp",
                                       name=R + f"tpsB{i}_{g}_{u}")
                        nc.tensor.matmul(psu[:], nat[:, s * P:(s + 1) * P],
                                         ident[:], start=True, stop=True)
                        dst = atp[(g, i)][:, u * P:(u + 1) * P]
                        if copy_mode == "dve" or (copy_mode == "both"
                                                  and g % 2 == 0):
                            nc.vector.tensor_copy(dst, psu[:])
                        else:
                            nc.scalar.copy(dst, psu[:])
                    continue
                ps = tpp.tile([P, 512], F32, tag="tp", name=R + f"tps{i}_{g}")
                for u in range(4):
                    s = q * 4 + u
                    nc.tensor.matmul(ps[:, u * P:(u + 1) * P],
                                     nat[:, s * P:(s + 1) * P], ident[:],
                                     start=True, stop=True)
                if copy_mode == "none":
                    continue
                if copy_mode == "dve" or (copy_mode == "both" and g % 3 != 2):
                    nc.vector.tensor_copy(atp[(g, i)][:], ps[:])
                else:
                    nc.scalar.copy(atp[(g, i)][:], ps[:])
        # degree -> d for this m-tile
        deg = dtp.tile([P, 1], F32, tag="deg", bufs=2, name=R + f"deg{i}")
        nc.vector.tensor_reduce(deg[:], par[:, i * NCH:(i + 1) * NCH],
                                axis=AxX, op=Alu.add)
        deg2 = dtp.tile([P, 1], F32, tag="deg2", bufs=2, name=R + f"deg2{i}")
        nc.vector.tensor_scalar_add(deg2[:], deg[:], lv[:])
        s0 = dtp.tile([P, 1], F32, tag="s0", bufs=2, name=R + f"s0{i}")
        nc.scalar.sqrt(s0[:], deg2[:])
        r0 = dtp.tile([P, 1], F32, tag="r0", bufs=2, name=R + f"r0{i}")
        nc.vector.reciprocal(r0[:], s0[:])
        # one Newton step: d = r0 * (1.5 - 0.5 * deg2 * r0^2)
        t1 = dtp.tile([P, 1], F32, tag="t1", bufs=2, name=R + f"t1{i}")
        nc.vector.tensor_mul(t1[:], r0[:], r0[:])
        t2 = dtp.tile([P, 1], F32, tag="t2", bufs=2, name=R + f"t2{i}")
        nc.vector.tensor_mul(t2[:], t1[:], deg2[:])
        t3 = dtp.tile([P, 1], F32, tag="t3", bufs=2, name=R + f"t3{i}")
        nc.vector.tensor_scalar(t3[:], t2[:], -0.5, 1.5, op0=Alu.mult,
                                op1=Alu.add)
        nc.vector.tensor_mul(dcols[:, i:i + 1], r0[:], t3[:])
        if (i + 1) % MG == 0:
            cc = (i + 1) // MG - 1    # chunk index
            if do_coll:
                nc.gpsimd.dma_start(dag_in[cc][:],
                                    dcols[:, cc * MG:(cc + 1) * MG])
                nc.gpsimd.collective_compute(
                    "AllGather", Alu.bypass, replica_groups=RG,
                    ins=[dag_in[cc].opt()], outs=[dag_out[cc].opt()])
                nc.gpsimd.dma_start(
                    dsl[cc][:],
                    dag_out[cc][:].rearrange("(r p m) o -> p r (m o)",
                                             r=N_CORES, p=P, m=MG))
            if do_sv:
                for r in range(N_CORES):
                    for mg in range(MG):
                        t = r * MT + cc * MG + mg
                        stile = supp.tile([P, DOUT], BF16, tag="sup",
                                          name=R + f"stile{t}")
                        nc.gpsimd.dma_start(stile[:],
                                            sag_out[t * P:(t + 1) * P, :])
                        nc.vector.tensor_scalar_mul(
                            sv[t][:], stile[:],
                            dsl[cc][:, r * MG + mg:r * MG + mg + 1])
            if fake_sv:
                for r in range(N_CORES):
                    for mg in range(MG):
                        t = r * MT + cc * MG + mg
                        stile = supp.tile([P, DOUT], BF16, tag="sup",
                                          name=R + f"stile{t}")
                        nc.gpsimd.dma_start(stile[:],
                                            sag_in[(t % MT) * P:(t % MT + 1) * P, :])
                        nc.vector.tensor_scalar_mul(
                            sv[t][:], stile[:], dcols[:, 0:1])

    # ---- main matmul: out[m, n] accumulated over k (AG-phase order) ----
    if not do_mm:
        return
    mmps = [mmp.tile([P, 512], F32, tag="mmps", name=R + f"mmps_{ii}")
            for ii in range(MT // 2)]
    for phase in range(MT):
        for r in range(N_CORES):
            t = r * MT + phase
            first = (phase == 0 and r == 0)
            last = (phase == MT - 1 and r == N_CORES - 1)
            # phase order already matches chunked-AG availability
            for i in range(MT):
                dst = mmps[i // 2][:, (i % 2) * DOUT:(i % 2 + 1) * DOUT]
                # start=True clears the WHOLE bank's has_written bits, so
                # only the bank's first matmul (even slice) may carry it;
                # the odd slice's first matmul overwrites via cleared bits.
                nc.tensor.matmul(
                    dst,
                    atp[(t // 4, i)][:, (t % 4) * P:(t % 4 + 1) * P],
                    sv[t][:],
                    start=first and (i % 2 == 0), stop=last,
                    skip_group_check=True)

    # ---- epilogue: scale rows by d_m, add bias, store ----
    for i in range(MT):
        src = mmps[i // 2][:, (i % 2) * DOUT:(i % 2 + 1) * DOUT]
        st1 = stagep.tile([P, DOUT], F32, tag="stage", name=R + f"st1_{i}")
        nc.vector.tensor_scalar_mul(st1[:], src, dcols[:, i:i + 1])
        st2 = stagep.tile([P, DOUT], F32, tag="stage", name=R + f"st2_{i}")
        nc.vector.tensor_add(st2[:], st1[:], bias_bc[:])
        nc.sync.dma_start(out.ap()[i * P:(i + 1) * P, :], st2[:])


def build(repeat=1, stage="full", dg=2):
    nc = bacc.Bacc("TRN2", target_bir_lowering=False, debug=False,
                   num_devices=N_CORES)
    a = nc.dram_tensor("a", [M_LOC, N], F32, kind="ExternalInput")
    x = nc.dram_tensor("x", [M_LOC, DIN], F32, kind="ExternalInput")
    w = nc.dram_tensor("w", [DIN, DOUT], F32, kind="ExternalInput")
    bias = nc.dram_tensor("bias", [DOUT], F32, kind="ExternalInput")
    lvec = nc.dram_tensor("lvec", [P, 1], F32, kind="ExternalInput")
    out = nc.dram_tensor("out", [M_LOC, DOUT], F32, kind="ExternalOutput")

    with tile.TileContext(nc) as tc, ExitStack() as ctx:
        cpool = ctx.enter_context(tc.tile_pool(name="cpool", bufs=1))
        natp = ctx.enter_context(tc.tile_pool(name="natp", bufs=2))
        natbp = ctx.enter_context(tc.tile_pool(name="natbp", bufs=3))
        supp = ctx.enter_context(tc.tile_pool(name="supp", bufs=8))
        xtp = ctx.enter_context(tc.tile_pool(name="xtp", bufs=2))
        atpp = ctx.enter_context(tc.tile_pool(name="atpp", bufs=GT * MT))
        svp = ctx.enter_context(tc.tile_pool(name="svp", bufs=KT))
        dslp = ctx.enter_context(tc.tile_pool(name="dslp", bufs=MT))
        dtp = ctx.enter_context(tc.tile_pool(name="dtp", bufs=1))
        stagep = ctx.enter_context(tc.tile_pool(name="stagep", bufs=2))
        tpp = ctx.enter_context(tc.tile_pool(name="tpp", bufs=4, space="PSUM"))
        mmp = ctx.enter_context(tc.tile_pool(name="mmp", bufs=MT // 2,
                                             space="PSUM"))
        dram = ctx.enter_context(tc.tile_pool(name="dram", bufs=1,
                                              space="DRAM"))

        # ---- constants ----
        ones_bf = cpool.tile([P, P], BF16)
        nc.vector.memset(ones_bf[:], 1.0)
        ident = cpool.tile([P, P], BF16)
        nc.gpsimd.affine_select(
            ident[:], ones_bf[:], pattern=[[1, P]],
            compare_op=Alu.is_equal, fill=0.0, base=0, channel_multiplier=-1)
        wb = []
        for dt in range(DIN // P):
            wt = cpool.tile([P, DOUT], BF16, tag=f"wb{dt}", name=f"wb{dt}")
            nc.gpsimd.dma_start(wt[:], w.ap()[dt * P:(dt + 1) * P, :])
            wb.append(wt)
        lv = cpool.tile([P, 1], F32, tag="lv")
        nc.scalar.dma_start(lv[:], lvec.ap())
        # broadcast bias over partitions with a K=1 matmul
        ones_row = cpool.tile([1, P], F32, tag="ones_row")
        nc.vector.memset(ones_row[:], 1.0)
        bias_row = cpool.tile([1, DOUT], F32, tag="bias_row")
        nc.scalar.dma_start(bias_row[:], bias.ap()[None, :])
        bias_bc = cpool.tile([P, DOUT], F32, tag="bias_bc")
        bps = tpp.tile([P, 512], F32, tag="tp", name="bias_ps")
        nc.tensor.matmul(bps[:, 0:DOUT], ones_row[:], bias_row[:],
                         start=True, stop=True)
        nc.vector.tensor_copy(bias_bc[:], bps[:, 0:DOUT])

        auxp = tpp
        pools = (natp, natbp, supp, xtp, atpp, svp, dslp, dtp, stagep,
                 tpp, mmp, auxp, dram)
        consts = (ident, wb, bias_bc, lv, a, x, w, bias, out)
        for rep in range(repeat):
            _emit_body(nc, tc, pools, consts, rep, stage=stage, dg=dg)
    nc.compile()
    return nc


def make_in_maps(adjacency, input_feature, weight, bias, l):
    adjacency = np.ascontiguousarray(np.asarray(adjacency, dtype=np.float32))
    input_feature = np.ascontiguousarray(
        np.asarray(input_feature, dtype=np.float32))
    weight = np.ascontiguousarray(np.asarray(weight, dtype=np.float32))
    bias_np = np.ascontiguousarray(np.asarray(bias, dtype=np.float32))
    lval = float(np.asarray(l))
    lv = np.full((P, 1), lval, dtype=np.float32)
    in_maps = []
    for c in range(N_CORES):
        in_maps.append({
            "a": adjacency[c * M_LOC:(c + 1) * M_LOC, :],
            "x": input_feature[c * M_LOC:(c + 1) * M_LOC, :],
            "w": weight,
            "bias": bias_np,
            "lvec": lv,
        })
    return in_maps


_NC_CACHE = None


def kernel(adjacency, input_feature, weight, bias, l):
    global _NC_CACHE
    if _NC_CACHE is None:
        _NC_CACHE = build()
    nc = _NC_CACHE
    in_maps = make_in_maps(adjacency, input_feature, weight, bias, l)
    res = bass_utils.run_bass_kernel_spmd(nc, in_maps,
                                          core_ids=list(range(N_CORES)))
    blocks = [res.results[c]["out"] for c in range(N_CORES)]
    return np.ascontiguousarray(np.concatenate(blocks, axis=0),
                                dtype=np.float32)


if __name__ == "__main__":
    rng = np.random.default_rng(0)
    A = rng.random((N, N), dtype=np.float32)
    X = rng.standard_normal((N, DIN)).astype(np.float32)
    W = (rng.standard_normal((DIN, DOUT)) / np.sqrt(DIN)).astype(np.float32)
    B = np.zeros((DOUT,), dtype=np.float32)
    out = kernel(A, X, W, B, 1)
    deg = A.sum(axis=1) + 1.0
    d = np.where(deg > 0, deg ** -0.5, 0.0).astype(np.float32)
    ref = (A * d[:, None] * d[None, :]) @ (X @ W) + B
    err = np.abs(out - ref)
    rel = np.linalg.norm(out - ref) / np.linalg.norm(ref)
    print(f"max abs err {err.max():.3e}  rel l2 {rel:.3e}")
